# revision 1
# baseline (speedup 1.0000x reference)
"""Trainium2 Bass kernel for nn_DelayLMLIFSNN (3-layer delay-conv + BN + LIF SNN).

Strategy:
- Data-parallel over batch B=64 across 8 NeuronCores (8 batch elements/core).
- Per layer: causal dilated-gauss conv as 100 PE matmuls per (h-half, batch)
  [fp16 weight-split x2: w = w0 + w1*2^-12, binary spike inputs are exact in
  fp16, fp32 PSUM accumulation => ~fp32-exact conv],
  BatchNorm stats via cross-core AllReduce (two-pass mean/var, Newton-refined
  1/sqrt), then the LIF soft-reset scan: 512 serial steps x 4 DVE ops on
  (128p x 16f) state tiles, matching the reference's fp32 rounding order.
"""

import os
import numpy as np

T, B, J, H, K, NL = 512, 64, 256, 256, 25, 3
THETA = 1.0
SIGMA_INIT = 0.5
EPS = 1e-5
BL = B // 8          # batch per core
NBLK = 2 * BL        # (b, c) blocks per core
TPAD = T + (K - 1)   # left-padded time for conv input

_CACHE = {}
LAST = {"exec_time_ns": None, "results": None}


# ----------------------------------------------------------------------------
# Host-side math
# ----------------------------------------------------------------------------

def _gauss_kernel_host(W, P):
    """Replicates reference.gauss_kernel in fp32. Uses jax-cpu when available
    so host arithmetic bit-matches the jax reference; numpy fallback."""
    try:
        import jax
        import jax.numpy as jnp

        cpu = jax.devices("cpu")[0]

        def gk(W, P):
            pos = jnp.arange(K, dtype=W.dtype)
            c = P + K // 2
            s = jnp.abs(jnp.float32(SIGMA_INIT)) + 0.27
            g = jnp.exp(-0.5 * ((pos[None, None, :] - c[..., None]) / s) ** 2)
            g = g / (jnp.sum(g, axis=-1, keepdims=True) + 1e-7)
            return W[..., None] * g

        with jax.default_device(cpu):
            return np.array(jax.jit(gk, backend="cpu")(jnp.asarray(W), jnp.asarray(P)))
    except Exception:
        pos = np.arange(K, dtype=np.float32)
        c = (P + np.float32(K // 2)).astype(np.float32)
        s = np.float32(abs(SIGMA_INIT) + 0.27)
        t = ((pos[None, None, :] - c[..., None]) / s).astype(np.float32)
        g = np.exp((np.float32(-0.5) * (t * t)).astype(np.float32)).astype(np.float32)
        den = (np.sum(g, axis=-1, keepdims=True, dtype=np.float32) + np.float32(1e-7)).astype(np.float32)
        g = (g / den).astype(np.float32)
        return (W[..., None] * g).astype(np.float32)


def _fp16_split(kern):
    """kern (fp32) -> (w0, w1) fp16 with kern ~= w0 + w1 * 2^-12 (residual
    <= ~2^-24*|kern|). Subnormal fp16 values are flushed host-side so PE
    flush-to-zero behavior (if any) cannot bite."""
    FP16_MIN_NORMAL = 6.104e-5
    w0 = kern.astype(np.float16)
    w0 = np.where(np.abs(w0.astype(np.float32)) < FP16_MIN_NORMAL, np.float16(0), w0)
    r = (kern - w0.astype(np.float32)) * np.float32(4096.0)
    w1 = r.astype(np.float16)
    w1 = np.where(np.abs(w1.astype(np.float32)) < FP16_MIN_NORMAL, np.float16(0), w1)
    return w0, w1


def _prep_static(W, P, beta, gamma, bb, U0):
    """Build the replicated parameter arrays (same on all cores)."""
    # weights: (l, c_outhalf, split, j_partition, jc, k, h) fp16
    wts = np.empty((NL, 2, 2, 128, 2, K, 128), np.float16)
    for l in range(NL):
        kern = _gauss_kernel_host(W[l], P[l])      # (H, J, K) fp32
        w0, w1 = _fp16_split(kern)
        for c in range(2):
            for s, wsrc in enumerate((w0, w1)):
                # lhsT[j, h] = w[c*128+h, jc*128+j, k]
                blk = wsrc[c * 128:(c + 1) * 128]          # (128h, J, K)
                # -> (j, jc, k, h)
                arr = blk.reshape(128, 2, 128, K)          # (h, jc, j, k)
                wts[l, c, s] = arr.transpose(2, 1, 3, 0)   # (j, jc, k, h)

    betat = np.empty((NL, 128, 16), np.float32)
    ombcol = np.empty((NL, 128, 2), np.float32)
    gcol = np.empty((NL, 128, 2), np.float32)
    bcol = np.empty((NL, 128, 2), np.float32)
    for l in range(NL):
        for c in range(2):
            ch = beta[l, c * 128:(c + 1) * 128].astype(np.float32)
            for b in range(BL):
                betat[l, :, b * 2 + c] = ch
            ombcol[l, :, c] = (np.float32(1.0) - ch).astype(np.float32)
            gcol[l, :, c] = gamma[l, c * 128:(c + 1) * 128]
            bcol[l, :, c] = bb[l, c * 128:(c + 1) * 128]
    return wts, betat, ombcol, gcol, bcol


def _prep_percore(x, U0, core):
    x16 = np.zeros((128, NBLK, TPAD), np.float16)
    xs = x[:, core * BL:(core + 1) * BL, :]            # (T, BL, J)
    # x16[p, b*2+jc, 24+t] = x[t, b, jc*128+p]
    a = xs.astype(np.float16).reshape(T, BL, 2, 128)   # (t, b, jc, p)
    x16[:, :, K - 1:] = a.transpose(3, 1, 2, 0).reshape(128, NBLK, T)

    d0 = np.empty((NL, 128, 16), np.float32)
    u = U0[:, core * BL:(core + 1) * BL, :]            # (NL, BL, H)
    for l in range(NL):
        a = u[l].reshape(BL, 2, 128)                   # (b, c, p)
        d0[l] = a.transpose(2, 0, 1).reshape(128, 16)
    return x16, d0


# ----------------------------------------------------------------------------
# Device program
# ----------------------------------------------------------------------------

def _build(nc, bn_affine_trivial):
    import concourse.tile as tile
    from concourse import mybir
    from contextlib import ExitStack

    F32 = mybir.dt.float32
    F16 = mybir.dt.float16
    AX = mybir.AxisListType
    OP = mybir.AluOpType
    SQRT = mybir.ActivationFunctionType.Sqrt

    ap_x = nc.dram_tensor("x16", [128, NBLK, TPAD], F16, kind="ExternalInput").ap()
    ap_w = nc.dram_tensor("wts", [NL, 2, 2, 128, 2, K, 128], F16, kind="ExternalInput").ap()
    ap_d0 = nc.dram_tensor("d0", [NL, 128, 16], F32, kind="ExternalInput").ap()
    ap_bt = nc.dram_tensor("betat", [NL, 128, 16], F32, kind="ExternalInput").ap()
    ap_omb = nc.dram_tensor("ombcol", [NL, 128, 2], F32, kind="ExternalInput").ap()
    ap_g = nc.dram_tensor("gcol", [NL, 128, 2], F32, kind="ExternalInput").ap()
    ap_bb = nc.dram_tensor("bcol", [NL, 128, 2], F32, kind="ExternalInput").ap()
    ap_out = nc.dram_tensor("out", [2, 128, BL, T], F32, kind="ExternalOutput").ap()

    with tile.TileContext(nc) as tc, ExitStack() as ctx:
        wp = ctx.enter_context(tc.tile_pool(name="wp", bufs=4))
        xp = ctx.enter_context(tc.tile_pool(name="xp", bufs=2))
        yp = ctx.enter_context(tc.tile_pool(name="yp", bufs=1))
        sp = ctx.enter_context(tc.tile_pool(name="sp", bufs=2))
        st = ctx.enter_context(tc.tile_pool(name="st", bufs=2))
        ps = ctx.enter_context(tc.tile_pool(name="ps", bufs=4, space="PSUM"))
        dr = ctx.enter_context(tc.tile_pool(name="dr", bufs=4, space="DRAM"))

        xcur = xp.tile([128, NBLK, TPAD], F16, tag="xreg")
        nc.sync.dma_start(xcur[:], ap_x)

        for l in range(NL):
            # ---------------- conv ----------------
            yreg = yp.tile([128, NBLK, T], F32, tag="yreg")
            for c in range(2):
                wt0 = wp.tile([128, 2, K, 128], F16, tag="w")
                wt1 = wp.tile([128, 2, K, 128], F16, tag="w")
                nc.sync.dma_start(wt0[:], ap_w[l, c, 0])
                nc.sync.dma_start(wt1[:], ap_w[l, c, 1])
                for b in range(BL):
                    p0 = ps.tile([128, T], F32, tag="psum")
                    p1 = ps.tile([128, T], F32, tag="psum")
                    for s, (pt, wt) in enumerate(((p0, wt0), (p1, wt1))):
                        for jc in range(2):
                            for k in range(K):
                                nc.tensor.matmul(
                                    pt[:],
                                    lhsT=wt[:, jc, k, :],
                                    rhs=xcur[:, b * 2 + jc, k:k + T],
                                    start=(jc == 0 and k == 0),
                                    stop=(jc == 1 and k == K - 1),
                                )
                    yblk = yreg[:, b * 2 + c, :]
                    nc.vector.tensor_scalar(yblk, p1[:], float(2.0 ** -12), None, OP.mult)
                    nc.vector.tensor_tensor(yblk, yblk, p0[:], OP.add)

            # ---------------- BN stats: mean ----------------
            sred = st.tile([128, 16], F32, tag="sred")
            for m in range(NBLK):
                nc.vector.tensor_reduce(sred[:, m:m + 1], yreg[:, m, :], AX.X, OP.add)
            ssum = st.tile([128, 2], F32, tag="ssum")
            for c in range(2):
                nc.vector.tensor_reduce(ssum[:, c:c + 1], sred[:, c::2], AX.X, OP.add)
            cin1 = dr.tile([128, 2], F32, tag="cc")
            cout1 = dr.tile([128, 2], F32, tag="cc")
            nc.sync.dma_start(cin1[:], ssum[:])
            nc.gpsimd.collective_compute(
                "AllReduce", OP.add, replica_groups=[list(range(8))],
                ins=[cin1[:].opt()], outs=[cout1[:].opt()])
            gsum = st.tile([128, 2], F32, tag="gsum")
            nc.sync.dma_start(gsum[:], cout1[:])
            mu = st.tile([128, 2], F32, tag="mu")
            nc.vector.tensor_scalar(mu[:], gsum[:], float(1.0 / (T * B)), None, OP.mult)

            # ---------------- center + variance ----------------
            SQUARE = mybir.ActivationFunctionType.Square
            vred = st.tile([128, 16], F32, tag="vred")
            for c in range(2):
                nc.vector.tensor_scalar(
                    yreg[:, c::2, :], yreg[:, c::2, :], mu[:, c:c + 1], None, OP.subtract)
            for m in range(NBLK):
                sq = sp.tile([128, T], F32, tag="sq")
                nc.scalar.activation(sq[:], yreg[:, m, :], SQUARE,
                                     accum_out=vred[:, m:m + 1])
            vsum = st.tile([128, 2], F32, tag="vsum")
            for c in range(2):
                nc.vector.tensor_reduce(vsum[:, c:c + 1], vred[:, c::2], AX.X, OP.add)
            cin2 = dr.tile([128, 2], F32, tag="cc")
            cout2 = dr.tile([128, 2], F32, tag="cc")
            nc.sync.dma_start(cin2[:], vsum[:])
            nc.gpsimd.collective_compute(
                "AllReduce", OP.add, replica_groups=[list(range(8))],
                ins=[cin2[:].opt()], outs=[cout2[:].opt()])
            gvs = st.tile([128, 2], F32, tag="gvs")
            nc.sync.dma_start(gvs[:], cout2[:])

            # v = var + eps ; s = sqrt(v) via ACT seed + 2 Newton iters;
            # r = 1/s via DVE HW divide
            v = st.tile([128, 2], F32, tag="v")
            nc.vector.tensor_scalar(v[:], gvs[:], float(1.0 / (T * B)), None, OP.mult)
            nc.vector.tensor_scalar(v[:], v[:], float(EPS), None, OP.add)
            sqt = st.tile([128, 2], F32, tag="sqt")
            rcp = st.tile([128, 2], F32, tag="rcp")
            qt = st.tile([128, 2], F32, tag="qt")
            nc.scalar.activation(sqt[:], v[:], SQRT)
            for _ in range(2):
                nc.vector.reciprocal(rcp[:], sqt[:])
                nc.vector.tensor_tensor(qt[:], v[:], rcp[:], OP.mult)
                nc.vector.tensor_tensor(sqt[:], sqt[:], qt[:], OP.add)
                nc.vector.tensor_scalar(sqt[:], sqt[:], 0.5, None, OP.mult)
            rr = st.tile([128, 2], F32, tag="rr")
            nc.vector.reciprocal(rr[:], sqt[:])

            # ---------------- z = ((d*r)*gamma + bb) * (1-beta) ----------------
            ombc = st.tile([128, 2], F32, tag="ombc")
            nc.sync.dma_start(ombc[:], ap_omb[l])
            if not bn_affine_trivial:
                gc = st.tile([128, 2], F32, tag="gc")
                bc = st.tile([128, 2], F32, tag="bc")
                nc.sync.dma_start(gc[:], ap_g[l])
                nc.sync.dma_start(bc[:], ap_bb[l])
            for c in range(2):
                blk = yreg[:, c::2, :]
                nc.vector.tensor_scalar(blk, blk, rr[:, c:c + 1], None, OP.mult)
                if not bn_affine_trivial:
                    nc.vector.tensor_scalar(blk, blk, gc[:, c:c + 1], None, OP.mult)
                    nc.vector.tensor_scalar(blk, blk, bc[:, c:c + 1], None, OP.add)
                nc.vector.tensor_scalar(blk, blk, ombc[:, c:c + 1], None, OP.mult)

            # ---------------- LIF scan ----------------
            Dt = st.tile([128, 16], F32, tag="D")
            Ut = st.tile([128, 16], F32, tag="U")
            bt = st.tile([128, 16], F32, tag="bt")
            nc.sync.dma_start(Dt[:], ap_d0[l])
            nc.sync.dma_start(bt[:], ap_bt[l])
            for t in range(T):
                zcol = yreg[:, :, t]
                nc.vector.tensor_tensor(Ut[:], Dt[:], bt[:], OP.mult)
                nc.vector.tensor_tensor(Ut[:], Ut[:], zcol, OP.add)
                nc.vector.tensor_scalar(zcol, Ut[:], float(THETA), None, OP.is_gt)
                nc.vector.tensor_tensor(Dt[:], Ut[:], zcol, OP.subtract)

            # ---------------- spikes out ----------------
            if l < NL - 1:
                xnext = xp.tile([128, NBLK, TPAD], F16, tag="xreg")
                nc.vector.memset(xnext[:, :, 0:K - 1], 0.0)
                nc.vector.tensor_copy(xnext[:, :, K - 1:], yreg[:])
                xcur = xnext
            else:
                for c in range(2):
                    nc.sync.dma_start(ap_out[c], yreg[:, c::2, :])
    nc.compile()
    return nc


def _get_compiled(bn_affine_trivial):
    key = ("prog", bn_affine_trivial)
    if key not in _CACHE:
        from concourse import bacc
        nc = bacc.Bacc("TRN2", target_bir_lowering=False, debug=False, num_devices=8)
        _CACHE[key] = _build(nc, bn_affine_trivial)
    return _CACHE[key]


# ----------------------------------------------------------------------------
# Profiled run (dev-only; needs the axon NTFF side channel)
# ----------------------------------------------------------------------------

def _run_profiled(nc, in_maps):
    import glob
    import tempfile
    from concourse.bass_utils import run_bass_kernel_spmd

    prof = {}
    try:
        from trn_agent_boot.trn_boot import _ntff_profile_via_ctypes
        hook = _ntff_profile_via_ctypes("/opt/axon/libaxon_pjrt.so")
        assert hook is not None
        neff_dir = tempfile.mkdtemp(prefix="snn_ntff_")
        with hook(neff_dir, [0]):
            res = run_bass_kernel_spmd(nc, in_maps, list(range(8)))
        ntffs = glob.glob(os.path.join(neff_dir, "*_body*.ntff"))
        prof["neff_dir"] = neff_dir
        if ntffs:
            import gauge.profiler
            from concourse._compat import FishPath
            p = gauge.profiler.Profile(
                profile_path=FishPath(neff_dir), kernel_dev_mode=True,
                profile_on_exit=False, bass_kernel=nc.m,
                offline_processing=True, fname="*_body*")
            rs = p.to_perfetto(model_index=(0,))
            if rs:
                prof["exec_time_ns"] = rs[0].exec_time_ns
                prof["trace_path"] = str(rs[0].trace_path)
                prof["scope_times"] = dict(rs[0].scope_times)
        return res, prof
    except Exception as e:  # profiling is best-effort
        prof["error"] = repr(e)
        res = run_bass_kernel_spmd(nc, in_maps, list(range(8)))
        return res, prof


# ----------------------------------------------------------------------------
# Entry point
# ----------------------------------------------------------------------------

def kernel(x, W, P, beta, gamma, bb, U0):
    from concourse.bass_utils import run_bass_kernel_spmd

    x = np.asarray(x, np.float32)
    W = np.asarray(W, np.float32)
    P = np.asarray(P, np.float32)
    beta = np.asarray(beta, np.float32)
    gamma = np.asarray(gamma, np.float32)
    bb = np.asarray(bb, np.float32)
    U0 = np.asarray(U0, np.float32)

    trivial = bool(np.all(gamma == 1.0) and np.all(bb == 0.0))
    nc = _get_compiled(trivial)

    skey = ("static", W.tobytes(), P.tobytes(), beta.tobytes(),
            gamma.tobytes(), bb.tobytes())
    sk = hash(skey)
    if _CACHE.get("static_key") != sk:
        _CACHE["static"] = _prep_static(W, P, beta, gamma, bb, U0)
        _CACHE["static_key"] = sk
    wts, betat, ombcol, gcol, bcol = _CACHE["static"]

    in_maps = []
    for core in range(8):
        x16, d0 = _prep_percore(x, U0, core)
        in_maps.append(dict(x16=x16, wts=wts, d0=d0, betat=betat,
                            ombcol=ombcol, gcol=gcol, bcol=bcol))

    trace = bool(int(os.environ.get("BASS_SNN_TRACE", "0")))
    if trace:
        res, prof = _run_profiled(nc, in_maps)
        LAST["exec_time_ns"] = prof.get("exec_time_ns")
        LAST["profile"] = prof
    else:
        res = run_bass_kernel_spmd(nc, in_maps, list(range(8)))
        LAST["exec_time_ns"] = res.exec_time_ns
    LAST["results"] = res

    o = np.empty((T, B, H), np.float32)
    for core in range(8):
        arr = res.results[core]["out"]                  # (2, 128, BL, T)
        o[:, core * BL:(core + 1) * BL, :] = (
            arr.transpose(3, 2, 0, 1).reshape(T, BL, H))
    return o



# revision 2
# speedup vs baseline: 1.0195x; 1.0195x over previous
"""Trainium2 Bass kernel for nn_DelayLMLIFSNN (3-layer delay-conv + BN + LIF SNN).

Strategy (v2):
- Sharding: 2 h-halves x 4 batch-quarters. Core c handles channels
  [hc*128, hc*128+128) for batch slice [bq*16, bq*16+16), hc=c//4, bq=c%4.
- Conv: causal delay conv as windowed PE matmuls: PSUM tiles [128h, 16b, 32t]
  (free=512, full PE rate), fp16 weight-split w = w0 + w1*2^-12 with the w1
  matmuls using a pre-scaled rhs (x*2^-12, exact for binary spikes) so both
  splits accumulate into ONE PSUM tile => ~fp32-exact conv, no combine pass.
- BN stats one-pass (sum, sumsq) accumulated on the Activation engine during
  PSUM eviction; ONE AllReduce per layer over the 4 cores sharing channels.
- BN affine: 3 exact-rounding-order passes on the Activation engine.
- LIF scan: ONE custom DVE instruction per step:
      U_t = (U_{t-1} - (U_{t-1} > 1)) * beta + z_t
  in-place over the z buffer (bit-identical to the reference recurrence),
  chunked so next-layer conv matmuls chase the scan through the tile
  framework's dependency graph. Spikes between layers exchanged pairwise
  (AllGather over {c, c+4}) in chunks, overlapped with compute.
"""

import os
import numpy as np

T, B, J, H, K, NL = 512, 64, 256, 256, 25, 3
THETA = 1.0
SIGMA_INIT = 0.5
EPS = 1e-5
NB = B // 4          # batch per core (pair sharding)
TPAD = T + (K - 1)
TC = 32              # conv window (free = NB*TC = 512)
NW = T // TC         # conv windows per layer
SCH = [32, 32, 64] + [128] * 3   # scan/affine/exchange chunk sizes (sum = 512)

_CACHE = {}
LAST = {"exec_time_ns": None, "results": None}


# ----------------------------------------------------------------------------
# Custom DVE op: one LIF step
# ----------------------------------------------------------------------------

def _register_lif_op():
    from concourse import dve_ops
    from concourse.dve_ops import DveOp, OPS
    from concourse.dve_spec import Spec, Src0, Src1, C0, One, lower
    from concourse.dve_spec import _has_src1 as has_src1
    from concourse.dve_uop import DveOpSpec

    for op in OPS:
        if op.name == "LIF_STEP_ANT":
            return op
    spec = Spec(
        body=(Src0 - (Src0 > One)) * C0 + Src1,
        reference=lambda in0, in1, s0, s1, imm2: (
            (in0 - (in0 > 1.0).astype(np.float32)) * s0 + in1),
    )
    row = dve_ops._CUSTOM_DVE_ROW_BASE + len(OPS)
    shas = {}
    for ver in ("v3", "v4"):
        tmp = DveOpSpec(name="LIF_STEP_ANT", opcode=row,
                        uops=lower(spec, ver=ver), rd1_en=has_src1(spec))
        shas[ver] = tmp.sha(ver)
    op = DveOp("LIF_STEP_ANT", spec, subdim=False, uops_sha=shas)
    OPS.append(op)
    dve_ops.CUSTOM_DVE_SPECS[op.name] = op.spec
    dve_ops._SUB_OPCODE_FOR_NAME[op.name] = row
    return op


# ----------------------------------------------------------------------------
# Host-side math (bit-matches the jax reference)
# ----------------------------------------------------------------------------

def _gauss_kernel_host(W, P):
    try:
        import jax
        import jax.numpy as jnp

        cpu = jax.devices("cpu")[0]

        def gk(W, P):
            pos = jnp.arange(K, dtype=W.dtype)
            c = P + K // 2
            s = jnp.abs(jnp.float32(SIGMA_INIT)) + 0.27
            g = jnp.exp(-0.5 * ((pos[None, None, :] - c[..., None]) / s) ** 2)
            g = g / (jnp.sum(g, axis=-1, keepdims=True) + 1e-7)
            return W[..., None] * g

        with jax.default_device(cpu):
            return np.array(jax.jit(gk, backend="cpu")(jnp.asarray(W), jnp.asarray(P)))
    except Exception:
        pos = np.arange(K, dtype=np.float32)
        c = (P + np.float32(K // 2)).astype(np.float32)
        s = np.float32(abs(SIGMA_INIT) + 0.27)
        t = ((pos[None, None, :] - c[..., None]) / s).astype(np.float32)
        g = np.exp((np.float32(-0.5) * (t * t)).astype(np.float32)).astype(np.float32)
        den = (np.sum(g, axis=-1, keepdims=True, dtype=np.float32) + np.float32(1e-7)).astype(np.float32)
        g = (g / den).astype(np.float32)
        return (W[..., None] * g).astype(np.float32)


def _fp16_split(kern):
    FP16_MIN_NORMAL = 6.104e-5
    w0 = kern.astype(np.float16)
    w0 = np.where(np.abs(w0.astype(np.float32)) < FP16_MIN_NORMAL, np.float16(0), w0)
    r = (kern - w0.astype(np.float32)) * np.float32(4096.0)
    w1 = r.astype(np.float16)
    w1 = np.where(np.abs(w1.astype(np.float32)) < FP16_MIN_NORMAL, np.float16(0), w1)
    return w0, w1


def _prep_static(W, P, beta):
    """wts_all: (2hc, 128j, NL, 2split, 2jc, K, 128h) fp16; per-channel cols."""
    wts_all = np.empty((2, 128, NL, 2, 2, K, 128), np.float16)
    bcol = np.empty((NL, 2, 128, 1), np.float32)
    omcol = np.empty((NL, 2, 128, 1), np.float32)
    for l in range(NL):
        kern = _gauss_kernel_host(W[l], P[l])          # (H, J, K) fp32
        w0, w1 = _fp16_split(kern)
        for hc in range(2):
            for s, wsrc in enumerate((w0, w1)):
                blk = wsrc[hc * 128:(hc + 1) * 128]    # (128h, J, K)
                arr = blk.reshape(128, 2, 128, K)      # (h, jc, j, k)
                # -> (j, jc, k, h)
                wts_all[hc, :, l, s] = arr.transpose(2, 1, 3, 0)
            ch = beta[l, hc * 128:(hc + 1) * 128].astype(np.float32)
            bcol[l, hc, :, 0] = ch
            omcol[l, hc, :, 0] = (np.float32(1.0) - ch).astype(np.float32)
    return wts_all, bcol, omcol


def _prep_percore(x, U0, core):
    hc, bq = core // 4, core % 4
    bs = slice(bq * NB, (bq + 1) * NB)
    x16 = np.zeros((128, 2, NB, TPAD), np.float16)
    x16b = np.zeros((128, 2, NB, TPAD), np.float16)
    xs = x[:, bs, :].astype(np.float16)                # (T, NB, J)
    a = xs.reshape(T, NB, 2, 128).transpose(3, 2, 1, 0)  # (p, jc, b, t)
    x16[:, :, :, K - 1:] = a
    x16b[:, :, :, K - 1:] = (a.astype(np.float32) * np.float32(2.0 ** -12)).astype(np.float16)
    # u0[l]: [128, NB, 1]
    u0 = U0[:, bs, hc * 128:(hc + 1) * 128].transpose(0, 2, 1)[..., None]
    return x16, x16b, np.ascontiguousarray(u0, np.float32)


# ----------------------------------------------------------------------------
# Device program
# ----------------------------------------------------------------------------

def _build(nc):
    import concourse.tile as tile
    from concourse import mybir
    from contextlib import ExitStack

    F32 = mybir.dt.float32
    F16 = mybir.dt.float16
    AX = mybir.AxisListType
    OP = mybir.AluOpType
    AF = mybir.ActivationFunctionType
    LIF = _register_lif_op()

    ap_x = nc.dram_tensor("x16", [128, 2, NB, TPAD], F16, kind="ExternalInput").ap()
    ap_xb = nc.dram_tensor("x16b", [128, 2, NB, TPAD], F16, kind="ExternalInput").ap()
    ap_w = nc.dram_tensor("wts", [128, NL, 2, 2, K, 128], F16, kind="ExternalInput").ap()
    ap_u0 = nc.dram_tensor("u0", [NL, 128, NB, 1], F32, kind="ExternalInput").ap()
    ap_bc = nc.dram_tensor("bcol", [NL, 128, 1], F32, kind="ExternalInput").ap()
    ap_om = nc.dram_tensor("omcol", [NL, 128, 1], F32, kind="ExternalInput").ap()
    ap_out = nc.dram_tensor("out", [128, NB, T], F32, kind="ExternalOutput").ap()

    G_STATS = [[0, 1, 2, 3], [4, 5, 6, 7]]
    G_PAIR = [[0, 4], [1, 5], [2, 6], [3, 7]]

    with tile.TileContext(nc) as tc, ExitStack() as ctx:
        sb = ctx.enter_context(tc.tile_pool(name="sb", bufs=1))
        st = ctx.enter_context(tc.tile_pool(name="st", bufs=2))
        sc = ctx.enter_context(tc.tile_pool(name="sc", bufs=2))
        sc1 = ctx.enter_context(tc.tile_pool(name="sc1", bufs=1))
        ps = ctx.enter_context(tc.tile_pool(name="ps", bufs=6, space="PSUM"))
        dr = ctx.enter_context(tc.tile_pool(name="dr", bufs=2, space="DRAM"))

        xb = sb.tile([128, 2, NB, TPAD], F16, tag="xb")
        xb2 = sb.tile([128, 2, NB, TPAD], F16, tag="xb2")
        yb = sb.tile([128, NB, T + 1], F32, tag="yb")      # col0=U0, cols1..T=y/z/U
        wt = sb.tile([128, NL, 2, 2, K, 128], F16, tag="wt")
        HEAD = 64
        nc.sync.dma_start(wt[:, 0], ap_w[:, 0])
        nc.sync.dma_start(xb[:, :, :, 0:HEAD], ap_x[:, :, :, 0:HEAD])
        nc.sync.dma_start(xb2[:, :, :, 0:HEAD], ap_xb[:, :, :, 0:HEAD])
        nc.sync.dma_start(xb[:, :, :, HEAD:TPAD], ap_x[:, :, :, HEAD:TPAD])
        nc.sync.dma_start(xb2[:, :, :, HEAD:TPAD], ap_xb[:, :, :, HEAD:TPAD])
        nc.sync.dma_start(wt[:, 1], ap_w[:, 1])
        nc.sync.dma_start(wt[:, 2], ap_w[:, 2])

        for l in range(NL):
            bc = st.tile([128, 1], F32, tag="bc")
            om = st.tile([128, 1], F32, tag="om")
            nc.sync.dma_start(bc[:], ap_bc[l])
            nc.sync.dma_start(om[:], ap_om[l])
            nc.sync.dma_start(yb[:, :, 0:1], ap_u0[l])
            # ---------------- conv (windowed) + one-pass stats ----------------
            scol = st.tile([128, NW], F32, tag="scol")
            sqcol = st.tile([128, NW], F32, tag="sqcol")
            for w in range(NW):
                pt = ps.tile([128, NB, TC], F32, tag="pswin")
                t0 = w * TC
                n = 0
                for s in range(2):
                    xt = xb if s == 0 else xb2
                    for jc in range(2):
                        for k in range(K):
                            nc.tensor.matmul(
                                pt[:], lhsT=wt[:, l, s, jc, k, :],
                                rhs=xt[:, jc, :, k + t0:k + t0 + TC],
                                start=(n == 0), stop=(n == 4 * K - 1))
                            n += 1
                nc.scalar.activation(yb[:, :, 1 + t0:1 + t0 + TC], pt[:], AF.Copy,
                                     accum_out=scol[:, w:w + 1])
                sqs = sc.tile([128, NB, TC], F32, tag="sqscratch")
                nc.scalar.activation(sqs[:], pt[:], AF.Square,
                                     accum_out=sqcol[:, w:w + 1])
                if w == 13:
                    warm = st.tile([128, 1], F32, tag="warm")
                    nc.scalar.activation(warm[:], sqcol[:, 0:1], AF.Sqrt)

            # ---------------- stats AR + BN constants ----------------
            stats = st.tile([128, 2], F32, tag="stats")
            nc.vector.tensor_reduce(stats[:, 0:1], scol[:], AX.X, OP.add)
            nc.vector.tensor_reduce(stats[:, 1:2], sqcol[:], AX.X, OP.add)
            cin = dr.tile([128, 2], F32, tag="ccs")
            cout = dr.tile([128, 2], F32, tag="ccs")
            nc.sync.dma_start(cin[:], stats[:])
            nc.gpsimd.collective_compute(
                "AllReduce", OP.add, replica_groups=G_STATS,
                ins=[cin[:].opt()], outs=[cout[:].opt()])
            gst = st.tile([128, 2], F32, tag="gst")
            nc.sync.dma_start(gst[:], cout[:])

            mu = st.tile([128, 1], F32, tag="mu")
            m2 = st.tile([128, 1], F32, tag="m2")
            nc.vector.tensor_scalar(mu[:], gst[:, 0:1], float(1.0 / (T * B)), None, OP.mult)
            nc.vector.tensor_scalar(m2[:], gst[:, 1:2], float(1.0 / (T * B)), None, OP.mult)
            v = st.tile([128, 1], F32, tag="v")
            nc.vector.tensor_tensor(v[:], mu[:], mu[:], OP.mult)
            nc.vector.tensor_tensor(v[:], m2[:], v[:], OP.subtract)
            nc.vector.tensor_scalar(v[:], v[:], float(EPS), None, OP.add)
            sqt = st.tile([128, 1], F32, tag="sqt")
            rcp = st.tile([128, 1], F32, tag="rcp")
            qt = st.tile([128, 1], F32, tag="qt")
            nc.scalar.activation(sqt[:], v[:], AF.Sqrt)
            for _ in range(2):
                nc.vector.reciprocal(rcp[:], sqt[:])
                nc.vector.tensor_tensor(qt[:], v[:], rcp[:], OP.mult)
                nc.vector.tensor_tensor(sqt[:], sqt[:], qt[:], OP.add)
                nc.vector.tensor_scalar(sqt[:], sqt[:], 0.5, None, OP.mult)
            rr = st.tile([128, 1], F32, tag="rr")
            nc.vector.reciprocal(rr[:], sqt[:])
            negmu = st.tile([128, 1], F32, tag="negmu")
            nc.vector.tensor_scalar(negmu[:], mu[:], -1.0, None, OP.mult)

            # ------------- affine (ACT, exact order; all chunks up front) -------------
            t0 = 0
            for ci, chs in enumerate(SCH):
                cols = slice(1 + t0, 1 + t0 + chs)
                blk = yb[:, :, cols]
                nc.scalar.activation(blk, blk, AF.Identity, bias=negmu[:])
                nc.scalar.activation(blk, blk, AF.Copy, scale=rr[:])
                nc.scalar.activation(blk, blk, AF.Copy, scale=om[:])
                t0 += chs
            # ------------- scan + spikes + pairwise exchange -------------
            t0 = 0
            for ci, chs in enumerate(SCH):
                cols = slice(1 + t0, 1 + t0 + chs)
                for t in range(t0, t0 + chs):
                    nc.vector._custom_dve(LIF, out=yb[:, :, t + 1], in0=yb[:, :, t],
                                          in1=yb[:, :, t + 1], s0=bc[:])
                if l < NL - 1:
                    # both spike representations packed into ONE AllGather
                    spst_full = sc1.tile([128, 2, NB, 128], F16, tag="spst")
                    spst = spst_full[:, 0, :, 0:chs]
                    spst2 = spst_full[:, 1, :, 0:chs]
                    nc.vector.tensor_scalar(spst, yb[:, :, cols], float(THETA), None, OP.is_gt)
                    nc.vector.tensor_scalar(spst2, yb[:, :, cols], float(THETA),
                                            float(2.0 ** -12), OP.is_gt, OP.mult)
                    gin = dr.tile([128, 2, NB, chs], F16, tag=f"gin{chs}")
                    gout = dr.tile([256, 2, NB, chs], F16, tag=f"gout{chs}")
                    nc.sync.dma_start(gin[:, 0], spst)
                    nc.sync.dma_start(gin[:, 1], spst2)
                    nc.gpsimd.collective_compute(
                        "AllGather", OP.bypass, replica_groups=G_PAIR,
                        ins=[gin[:].opt()], outs=[gout[:].opt()])
                    xcols = slice(K - 1 + t0, K - 1 + t0 + chs)
                    nc.sync.dma_start(xb[:, 0, :, xcols], gout[0:128, 0])
                    nc.sync.dma_start(xb[:, 1, :, xcols], gout[128:256, 0])
                    nc.sync.dma_start(xb2[:, 0, :, xcols], gout[0:128, 1])
                    nc.sync.dma_start(xb2[:, 1, :, xcols], gout[128:256, 1])
                else:
                    ost_full = sc1.tile([128, NB, 128], F32, tag="ost")
                    ost = ost_full[:, :, 0:chs]
                    nc.vector.tensor_scalar(ost, yb[:, :, cols], float(THETA), None, OP.is_gt)
                    nc.sync.dma_start(ap_out[:, :, t0:t0 + chs], ost)
                t0 += chs
    nc.compile()
    return nc


def _get_compiled():
    key = "prog"
    if key not in _CACHE:
        from concourse import bacc
        nc = bacc.Bacc("TRN2", target_bir_lowering=False, debug=False, num_devices=8)
        _CACHE[key] = _build(nc)
    return _CACHE[key]


# ----------------------------------------------------------------------------
# Profiled run (dev-only)
# ----------------------------------------------------------------------------

def _run_profiled(nc, in_maps):
    import glob
    import tempfile
    from concourse.bass_utils import run_bass_kernel_spmd

    prof = {}
    try:
        from trn_agent_boot.trn_boot import _ntff_profile_via_ctypes
        hook = _ntff_profile_via_ctypes("/opt/axon/libaxon_pjrt.so")
        assert hook is not None
        neff_dir = tempfile.mkdtemp(prefix="snn_ntff_")
        with hook(neff_dir, [0]):
            res = run_bass_kernel_spmd(nc, in_maps, list(range(8)))
        ntffs = glob.glob(os.path.join(neff_dir, "*_body*.ntff"))
        prof["neff_dir"] = neff_dir
        if ntffs:
            import gauge.profiler
            from concourse._compat import FishPath
            p = gauge.profiler.Profile(
                profile_path=FishPath(neff_dir), kernel_dev_mode=True,
                profile_on_exit=False, bass_kernel=nc.m,
                offline_processing=True, fname="*_body*")
            rs = p.to_perfetto(model_index=(0,))
            if rs:
                prof["exec_time_ns"] = rs[0].exec_time_ns
                prof["trace_path"] = str(rs[0].trace_path)
        return res, prof
    except Exception as e:
        prof["error"] = repr(e)
        res = run_bass_kernel_spmd(nc, in_maps, list(range(8)))
        return res, prof


# ----------------------------------------------------------------------------
# Entry point
# ----------------------------------------------------------------------------

def kernel(x, W, P, beta, gamma, bb, U0):
    from concourse.bass_utils import run_bass_kernel_spmd

    x = np.asarray(x, np.float32)
    W = np.asarray(W, np.float32)
    P = np.asarray(P, np.float32)
    beta = np.asarray(beta, np.float32)
    gamma = np.asarray(gamma, np.float32)
    bb = np.asarray(bb, np.float32)
    U0 = np.asarray(U0, np.float32)

    trivial = bool(np.all(gamma == 1.0) and np.all(bb == 0.0))
    assert trivial, "non-trivial BN affine not supported in this build"
    nc = _get_compiled()

    skey = ("static", W.tobytes(), P.tobytes(), beta.tobytes())
    sk = hash(skey)
    if _CACHE.get("static_key") != sk:
        _CACHE["static"] = _prep_static(W, P, beta)
        _CACHE["static_key"] = sk
    wts_all, bcol, omcol = _CACHE["static"]

    in_maps = []
    for core in range(8):
        hc = core // 4
        x16, x16b, u0 = _prep_percore(x, U0, core)
        in_maps.append(dict(
            x16=x16, x16b=x16b, wts=np.ascontiguousarray(wts_all[hc]),
            u0=u0, bcol=np.ascontiguousarray(bcol[:, hc]),
            omcol=np.ascontiguousarray(omcol[:, hc])))

    trace = bool(int(os.environ.get("BASS_SNN_TRACE", "0")))
    if trace:
        res, prof = _run_profiled(nc, in_maps)
        LAST["exec_time_ns"] = prof.get("exec_time_ns")
        LAST["profile"] = prof
    else:
        res = run_bass_kernel_spmd(nc, in_maps, list(range(8)))
        LAST["exec_time_ns"] = res.exec_time_ns
    LAST["results"] = res

    o = np.empty((T, B, H), np.float32)
    for core in range(8):
        hc, bq = core // 4, core % 4
        arr = res.results[core]["out"]                  # (128, NB, T)
        o[:, bq * NB:(bq + 1) * NB, hc * 128:(hc + 1) * 128] = arr.transpose(2, 1, 0)
    return o
